# revision 3
# baseline (speedup 1.0000x reference)
"""Conv2d 3x3 (pad=1, stride=1) on 8 TRN2 NeuronCores.

Input  (32, 128, 56, 56) f32, weight (256, 128, 3, 3) f32 -> out (32, 256, 56, 56) f32.

Strategy: data-parallel over batch (4 images per core). Per core, implicit GEMM:
for each of the 9 (kh, kw) taps, a [Cin=128 x Cout=128] stationary matmul against
a shifted window of the zero-padded input streams 448 output pixels (8 rows x 56)
per call, accumulating all 9 taps into one PSUM bank (fp32). Matmul operands are
fp16 (full-rate streaming, LDWEIGHTS hidden; ~3e-4 rel err from input
quantization, fp32 accumulate). float32r (~1.5e-4, ~8% slower) and float32
(~1e-7, 4x slower) remain available via MM_DT.
"""

import sys

sys.path.insert(0, "/opt/trn_rl_repo")

import numpy as np

import concourse.bass as bass  # noqa: F401
import concourse.mybir as mybir
import concourse.tile as tile
from concourse import bacc
from concourse.bass_utils import run_bass_kernel_spmd

B, CIN, H, W = 32, 128, 56, 56
COUT, KH, KW = 256, 3, 3
NCORES = 8
BPC = B // NCORES  # images per core
HP, WP = H + 2, W + 2  # zero-padded
NPIX = H * W  # 3136
ROWS_PER_TILE = 8
NT = H // ROWS_PER_TILE  # 7 free-dim tiles of 448
NFREE = ROWS_PER_TILE * W  # 448

MM_DT = mybir.dt.float16
Y_DT = mybir.dt.float16  # output stored fp16, cast to f32 on host (exact)
F32 = mybir.dt.float32

WARMUP_MMS = 8

# wstat mode: accumulate in the PADDED output domain so every matmul rhs is
# one contiguous slab of the flat padded input. Output pixel (h, w) lives at
# flat q = h*WP + w; tap (kh, kw) contributes xflat[q + kh*WP + kw]. Junk
# columns (w in {56, 57}) are dropped on the host. 7 row-tiles of 8 rows:
# rhs slab for (t, kh, kw) = xflat[:, 8*t*WP + kh*WP + kw :][:464].
PFREE = 8 * WP  # 464 per tile, fits a 2KB PSUM bank (512 f32)
YPIX = H * WP  # 3248 padded output pixels per image
XCOLS = HP * WP + 4  # tail pad: t=6, kh=kw=2 reads up to 3366


def dedupe_ldweights(nc, verbose: bool = False):
    """Remove InstLdweights that reload the PE array with the exact weights
    already resident (same AP/tile_position/perf_mode), post-tile-legalize.

    tile_legalize splits every InstMatmult into Ldweights+Matmult with no
    redundancy elimination, so a weight-stationary group of G matmuls pays G
    serialized ~107ns array loads instead of 1. Only Ldweights with no
    semaphore waits/updates are dropped (sync edges must be preserved), and
    tracking resets at every non-Ldweights PE-array-state hazard
    (self-loading or transpose matmuls) and at block boundaries.
    """
    removed = kept = synced = 0
    for f in nc.m.functions:
        for b in f.blocks:
            insts = list(b.instructions)
            out = []
            last_w = None
            changed = False
            for i in insts:
                if isinstance(i, mybir.InstLdweights):
                    si = i.sync_info
                    has_sync = bool(si and (si.on_wait or si.on_update))
                    key = (i.ins[0].concise(), str(i.tile_position),
                           str(i.perf_mode), str(i.is_transpose))
                    if key == last_w and not has_sync:
                        removed += 1
                        changed = True
                        continue
                    if key == last_w:
                        synced += 1
                    last_w = key
                    kept += 1
                elif isinstance(i, mybir.InstMatmult):
                    if i.ldweights or i.is_transpose:
                        last_w = None
                out.append(i)
            if changed:
                b.instructions = out
    if verbose:
        print(f"dedupe_ldweights: removed={removed} kept={kept} "
              f"(kept-redundant-with-sync={synced})")
    return removed


def build_conv_bass(reps: int = 1, warmup: int = WARMUP_MMS, xbufs: int = 4,
                    hint: bool = False, mm_dt=None, ydge: str = "scalar",
                    ncores: int = NCORES, y_dt=None, dma_mode: str = "split",
                    strip: str = "", mode: str = "base", dedup: bool = True,
                    verbose: bool = False):
    mm_dt = mm_dt or MM_DT
    y_dt = y_dt or Y_DT
    np_dt = mybir.dt.np(mm_dt)
    wstat = mode == "wstat"
    xcols = XCOLS if wstat else HP * WP
    ypix = YPIX if wstat else NPIX
    nc = bacc.Bacc("TRN2", target_bir_lowering=False, debug=False, num_devices=ncores)
    x = nc.dram_tensor("x", [BPC, CIN, xcols], mm_dt, kind="ExternalInput").ap()
    w = nc.dram_tensor("w", [CIN, KH * KW * COUT], mm_dt, kind="ExternalInput").ap()
    y = nc.dram_tensor("y", [BPC, COUT, ypix], y_dt, kind="ExternalOutput").ap()
    strip_x = "x" in strip
    strip_y = "y" in strip

    with tile.TileContext(nc) as tc:
        with (
            tc.tile_pool(name="wp", bufs=1) as wp,
            tc.tile_pool(name="xp", bufs=xbufs) as xp,
            tc.tile_pool(name="op", bufs=2) as op,
            tc.tile_pool(name="pp", bufs=8, space="PSUM") as pp,
        ):
            w_sb = wp.tile([CIN, KH * KW * COUT], mm_dt)
            nc.sync.dma_start(w_sb[:], w)

            if warmup:
                # Warm the PE HAM clock gate while the first input DMA is in
                # flight: memset a scratch tile (no DMA dependency), then spin
                # matmuls on it into a scratch PSUM bank that is never read.
                scratch = wp.tile([128, 640], F32)
                nc.vector.memset(scratch[:], 0.0)
                sc = scratch[:].bitcast(mm_dt)  # >= [128, 640] for any dtype <= 4B
                ps_warm = pp.tile([128, 512], F32, name="ps_warm", tag="ps")
                for _ in range(warmup):
                    nc.tensor.matmul(ps_warm[:], sc[:, :128], sc[:, 128:640],
                                     start=True, stop=True)

            # x DMA split: rows [0, 34) cover tiles t=0..3 (rows t*8 .. t*8+9);
            # rows [34, 58) cover t=4..6. Tile tracks sub-tile ranges, so the
            # first matmuls start as soon as the first chunk lands.
            XSPLIT = (35 if wstat else 34) * WP

            def body_wstat():
                # Weight-stationary: taps outer, row-tiles inner. Each of the
                # 7 PSUM-resident tiles accumulates all 9 taps; weights swap
                # only at tap boundaries (consecutive same-weight matmuls
                # stream at the stationary rate). Two half-groups (4+3 tiles)
                # so copies/DMA of one overlap matmuls of the next.
                for n in range(BPC):
                    if strip_x:
                        x_sb = x_static[n]
                    else:
                        x_sb = xp.tile([CIN, xcols], mm_dt)
                        nc.sync.dma_start(x_sb[:, :XSPLIT], x[n, :, :XSPLIT])
                        nc.sync.dma_start(x_sb[:, XSPLIT:], x[n, :, XSPLIT:])
                    for m in range(COUT // 128):
                        o_sb = op.tile([128, ypix], y_dt)
                        ydma = (nc.scalar.dma_start if ydge == "scalar"
                                else nc.sync.dma_start)
                        for ts in (range(0, 4), range(4, NT)):
                            ps = {t: pp.tile([128, PFREE], F32, name="ps",
                                             tag="ps") for t in ts}
                            for kh in range(KH):
                                for kw in range(KW):
                                    khw = kh * KW + kw
                                    lhsT = w_sb[:, khw * COUT + m * 128 : khw * COUT + m * 128 + 128]
                                    for t in ts:
                                        s = (8 * t + kh) * WP + kw
                                        nc.tensor.matmul(
                                            ps[t][:], lhsT, x_sb[:, s : s + PFREE],
                                            start=(khw == 0), stop=(khw == KH * KW - 1),
                                        )
                            for t in ts:
                                nc.vector.tensor_copy(
                                    o_sb[:, t * PFREE : (t + 1) * PFREE], ps[t][:])
                            if not strip_y:
                                lo, hi = ts[0] * PFREE, (ts[-1] + 1) * PFREE
                                ydma(y[n, m * 128 : (m + 1) * 128, lo:hi],
                                     o_sb[:, lo:hi])
                        last_o.append(o_sb)

            last_o = []
            x_static = None
            if strip_x:
                x_static = []
                for n in range(BPC):
                    xs = xp.tile([CIN, xcols], mm_dt)
                    nc.sync.dma_start(xs[:], x[n])
                    x_static.append(xs)

            def body():
                for n in range(BPC):
                    if strip_x:
                        x_sb = x_static[n]
                    else:
                        x_sb = xp.tile([CIN, HP * WP], mm_dt)
                        if dma_mode == "split":
                            nc.sync.dma_start(x_sb[:, :XSPLIT], x[n, :, :XSPLIT])
                            nc.sync.dma_start(x_sb[:, XSPLIT:], x[n, :, XSPLIT:])
                        else:
                            nc.sync.dma_start(x_sb[:], x[n])
                    xv = x_sb[:].rearrange("p (h w) -> p h w", h=HP)
                    for m in range(COUT // 128):
                        o_sb = op.tile([128, NPIX], y_dt)
                        for t in range(NT):
                            ps = pp.tile([128, NFREE], F32, name="ps", tag="ps")
                            for kh in range(KH):
                                for kw in range(KW):
                                    khw = kh * KW + kw
                                    lhsT = w_sb[:, khw * COUT + m * 128 : khw * COUT + m * 128 + 128]
                                    rhs = xv[:, t * ROWS_PER_TILE + kh : t * ROWS_PER_TILE + kh + ROWS_PER_TILE, kw : kw + W]
                                    nc.tensor.matmul(
                                        ps[:], lhsT, rhs,
                                        start=(khw == 0), stop=(khw == KH * KW - 1),
                                    )
                            nc.vector.tensor_copy(o_sb[:, t * NFREE : (t + 1) * NFREE], ps[:])
                            # store as soon as a half-phase of copies is done
                            ydma = nc.scalar.dma_start if ydge == "scalar" else nc.sync.dma_start
                            if t == 3 and dma_mode == "split" and not strip_y:
                                ydma(
                                    y[n, m * 128 : (m + 1) * 128, : 4 * NFREE],
                                    o_sb[:, : 4 * NFREE],
                                )
                        if strip_y:
                            pass
                        elif dma_mode == "split":
                            ydma(
                                y[n, m * 128 : (m + 1) * 128, 4 * NFREE :],
                                o_sb[:, 4 * NFREE :],
                            )
                        else:
                            ydma(y[n, m * 128 : (m + 1) * 128, :], o_sb[:])
                last_o.append(o_sb)

            thebody = body_wstat if wstat else body
            if reps == 1:
                thebody()
            else:
                with tc.For_i(0, reps, 1,
                              hint_engines=(mybir.EngineType.PE,) if hint else ()):
                    thebody()
            if strip_y:
                nc.scalar.dma_start(y[0, :128, :], last_o[-1][:])
    if dedup:
        dedupe_ldweights(nc, verbose=verbose)
    nc.compile()
    nc._np_mm_dt = np_dt
    nc._mode = mode
    return nc


MODE = "wstat"

_NC_CACHE = None


def _get_nc():
    global _NC_CACHE
    if _NC_CACHE is None:
        _NC_CACHE = build_conv_bass(mode=MODE)
    return _NC_CACHE


def run_conv(inputs: np.ndarray, weight: np.ndarray, nc=None, **spmd_kwargs):
    """Returns (output, BassKernelResults)."""
    x = np.ascontiguousarray(np.asarray(inputs, dtype=np.float32))
    w = np.asarray(weight, dtype=np.float32)
    assert x.shape == (B, CIN, H, W) and w.shape == (COUT, CIN, KH, KW)

    nc = nc or _get_nc()
    np_dt = nc._np_mm_dt
    wstat = getattr(nc, "_mode", "base") == "wstat"

    xcols = XCOLS if wstat else HP * WP
    xpad = np.zeros((B, CIN, xcols), np_dt)
    xpad[:, :, : HP * WP].reshape(B, CIN, HP, WP)[:, :, 1 : H + 1, 1 : W + 1] = x
    # w_r[cin, khw*COUT + cout] = weight[cout, cin, kh, kw]
    wr = np.ascontiguousarray(
        w.transpose(1, 2, 3, 0).reshape(CIN, KH * KW * COUT).astype(np_dt)
    )

    in_maps = [
        {"x": xpad[c * BPC : (c + 1) * BPC], "w": wr} for c in range(NCORES)
    ]
    r = run_bass_kernel_spmd(nc, in_maps, core_ids=list(range(NCORES)), **spmd_kwargs)
    if wstat:
        out = np.concatenate(
            [np.asarray(r.results[c]["y"]).astype(np.float32)
             .reshape(BPC, COUT, H, WP)[:, :, :, :W] for c in range(NCORES)],
            axis=0,
        )
    else:
        out = np.concatenate(
            [np.asarray(r.results[c]["y"]).astype(np.float32).reshape(BPC, COUT, H, W)
             for c in range(NCORES)], axis=0
        )
    return np.ascontiguousarray(out), r


def kernel(inputs: np.ndarray, weight: np.ndarray) -> np.ndarray:
    out, _ = run_conv(inputs, weight)
    return out



# revision 7
# speedup vs baseline: 1.3605x; 1.3605x over previous
"""Conv2d 3x3 (pad=1, stride=1) on 8 TRN2 NeuronCores.

Input  (32, 128, 56, 56) f32, weight (256, 128, 3, 3) f32 -> out (32, 256, 56, 56) f32.

Strategy: data-parallel over batch (4 images per core). Per core, implicit GEMM:
for each of the 9 (kh, kw) taps, a [Cin=128 x Cout=128] stationary matmul against
a shifted window of the zero-padded input streams 448 output pixels (8 rows x 56)
per call, accumulating all 9 taps into one PSUM bank (fp32). Matmul operands are
fp16 (~3.6e-4 rel err from input quantization, fp32 accumulate). bf16 is the
same speed (2.4e-3 rel err); float32r ~2x slower; float32 4x slower (MM_DT).

Two hard-won facts (this machine):
- tile_legalize emits one LDWEIGHTS per MATMUL with no redundancy
  elimination; un-hidden that costs ~107ns per 448-col matmul (~35%).
  dedupe_ldweights() removes redundant loads post-legalize (weights are
  grouped tap-outer so 3-4 consecutive matmuls share the stationary
  operand); measured best-state per-iter dropped 151.6us -> 101.6us.
- mode="ws448" streams only the 448 real pixels per row-tile via strided
  3D APs (vs "wstat"'s 464-wide contiguous padded slabs): 94.1us PE floor.
  Sim (CoreSim no_exec): 103.1us single-shot, PE 94% busy.
"""

import sys

sys.path.insert(0, "/opt/trn_rl_repo")

import numpy as np

import concourse.bass as bass  # noqa: F401
import concourse.mybir as mybir
import concourse.tile as tile
from concourse import bacc
from concourse.bass_utils import run_bass_kernel_spmd

B, CIN, H, W = 32, 128, 56, 56
COUT, KH, KW = 256, 3, 3
NCORES = 8
BPC = B // NCORES  # images per core
HP, WP = H + 2, W + 2  # zero-padded
NPIX = H * W  # 3136
ROWS_PER_TILE = 8
NT = H // ROWS_PER_TILE  # 7 free-dim tiles of 448
NFREE = ROWS_PER_TILE * W  # 448

MM_DT = mybir.dt.float16
Y_DT = mybir.dt.float16  # output stored fp16, cast to f32 on host (exact)
F32 = mybir.dt.float32

WARMUP_MMS = 8

# wstat mode: accumulate in the PADDED output domain so every matmul rhs is
# one contiguous slab of the flat padded input. Output pixel (h, w) lives at
# flat q = h*WP + w; tap (kh, kw) contributes xflat[q + kh*WP + kw]. Junk
# columns (w in {56, 57}) are dropped on the host. 7 row-tiles of 8 rows:
# rhs slab for (t, kh, kw) = xflat[:, 8*t*WP + kh*WP + kw :][:464].
PFREE = 8 * WP  # 464 per tile, fits a 2KB PSUM bank (512 f32)
YPIX = H * WP  # 3248 padded output pixels per image
XCOLS = HP * WP + 4  # tail pad: t=6, kh=kw=2 reads up to 3366


def dedupe_ldweights(nc, verbose: bool = False):
    """Remove InstLdweights that reload the PE array with the exact weights
    already resident (same AP/tile_position/perf_mode), post-tile-legalize.

    tile_legalize splits every InstMatmult into Ldweights+Matmult with no
    redundancy elimination, so a weight-stationary group of G matmuls pays G
    serialized ~107ns array loads instead of 1. Only Ldweights with no
    semaphore waits/updates are dropped (sync edges must be preserved), and
    tracking resets at every non-Ldweights PE-array-state hazard
    (self-loading or transpose matmuls) and at block boundaries.
    """
    removed = kept = synced = 0
    for f in nc.m.functions:
        for b in f.blocks:
            insts = list(b.instructions)
            out = []
            last_w = None
            changed = False
            for i in insts:
                if isinstance(i, mybir.InstLdweights):
                    si = i.sync_info
                    has_sync = bool(si and (si.on_wait or si.on_update))
                    key = (i.ins[0].concise(), str(i.tile_position),
                           str(i.perf_mode), str(i.is_transpose))
                    if key == last_w and not has_sync:
                        removed += 1
                        changed = True
                        continue
                    if key == last_w:
                        synced += 1
                    last_w = key
                    kept += 1
                elif isinstance(i, mybir.InstMatmult):
                    if i.ldweights or i.is_transpose:
                        last_w = None
                out.append(i)
            if changed:
                b.instructions = out
    if verbose:
        print(f"dedupe_ldweights: removed={removed} kept={kept} "
              f"(kept-redundant-with-sync={synced})")
    return removed


def build_conv_bass(reps: int = 1, warmup: int = WARMUP_MMS, xbufs: int = 4,
                    hint: bool = False, mm_dt=None, ydge: str = "scalar",
                    ncores: int = NCORES, y_dt=None, dma_mode: str = "split",
                    strip: str = "", mode: str = "base", dedup: bool = True,
                    verbose: bool = False):
    mm_dt = mm_dt or MM_DT
    y_dt = y_dt or Y_DT
    np_dt = mybir.dt.np(mm_dt)
    wstat = mode == "wstat"
    xcols = XCOLS if wstat else HP * WP
    ypix = YPIX if wstat else NPIX
    nc = bacc.Bacc("TRN2", target_bir_lowering=False, debug=False, num_devices=ncores)
    x = nc.dram_tensor("x", [BPC, CIN, xcols], mm_dt, kind="ExternalInput").ap()
    w = nc.dram_tensor("w", [CIN, KH * KW * COUT], mm_dt, kind="ExternalInput").ap()
    y = nc.dram_tensor("y", [BPC, COUT, ypix], y_dt, kind="ExternalOutput").ap()
    strip_x = "x" in strip
    strip_y = "y" in strip

    with tile.TileContext(nc) as tc:
        with (
            tc.tile_pool(name="wp", bufs=1) as wp,
            tc.tile_pool(name="xp", bufs=xbufs) as xp,
            tc.tile_pool(name="op", bufs=2) as op,
            tc.tile_pool(name="pp", bufs=8, space="PSUM") as pp,
        ):
            w_sb = wp.tile([CIN, KH * KW * COUT], mm_dt)
            nc.sync.dma_start(w_sb[:], w)

            if warmup:
                # Warm the PE HAM clock gate while the first input DMA is in
                # flight: memset a scratch tile (no DMA dependency), then spin
                # matmuls on it into a scratch PSUM bank that is never read.
                scratch = wp.tile([128, 640], F32)
                nc.vector.memset(scratch[:], 0.0)
                sc = scratch[:].bitcast(mm_dt)  # >= [128, 640] for any dtype <= 4B
                ps_warm = pp.tile([128, 512], F32, name="ps_warm", tag="ps")
                for _ in range(warmup):
                    nc.tensor.matmul(ps_warm[:], sc[:, :128], sc[:, 128:640],
                                     start=True, stop=True)

            # x DMA split: rows [0, 34) cover tiles t=0..3 (rows t*8 .. t*8+9);
            # rows [34, 58) cover t=4..6. Tile tracks sub-tile ranges, so the
            # first matmuls start as soon as the first chunk lands.
            XSPLIT = (35 if wstat else 34) * WP

            def body_wstat():
                # Weight-stationary: taps outer, row-tiles inner. Each of the
                # 7 PSUM-resident tiles accumulates all 9 taps; weights swap
                # only at tap boundaries (consecutive same-weight matmuls
                # stream at the stationary rate). Two half-groups (4+3 tiles)
                # so copies/DMA of one overlap matmuls of the next.
                for n in range(BPC):
                    if strip_x:
                        x_sb = x_static[n]
                    else:
                        x_sb = xp.tile([CIN, xcols], mm_dt)
                        nc.sync.dma_start(x_sb[:, :XSPLIT], x[n, :, :XSPLIT])
                        nc.sync.dma_start(x_sb[:, XSPLIT:], x[n, :, XSPLIT:])
                    for m in range(COUT // 128):
                        o_sb = op.tile([128, ypix], y_dt)
                        ydma = (nc.scalar.dma_start if ydge == "scalar"
                                else nc.sync.dma_start)
                        for ts in (range(0, 4), range(4, NT)):
                            ps = {t: pp.tile([128, PFREE], F32, name="ps",
                                             tag="ps") for t in ts}
                            for kh in range(KH):
                                for kw in range(KW):
                                    khw = kh * KW + kw
                                    lhsT = w_sb[:, khw * COUT + m * 128 : khw * COUT + m * 128 + 128]
                                    for t in ts:
                                        s = (8 * t + kh) * WP + kw
                                        nc.tensor.matmul(
                                            ps[t][:], lhsT, x_sb[:, s : s + PFREE],
                                            start=(khw == 0), stop=(khw == KH * KW - 1),
                                        )
                            for t in ts:
                                nc.vector.tensor_copy(
                                    o_sb[:, t * PFREE : (t + 1) * PFREE], ps[t][:])
                            if not strip_y:
                                lo, hi = ts[0] * PFREE, (ts[-1] + 1) * PFREE
                                ydma(y[n, m * 128 : (m + 1) * 128, lo:hi],
                                     o_sb[:, lo:hi])
                        last_o.append(o_sb)

            last_o = []
            x_static = None
            if strip_x:
                x_static = []
                for n in range(BPC):
                    xs = xp.tile([CIN, xcols], mm_dt)
                    nc.sync.dma_start(xs[:], x[n])
                    x_static.append(xs)

            def body_ws448():
                # Weight-stationary like body_wstat (taps outer within a
                # half-group, dedupe collapses the per-tap LDWEIGHTS), but
                # with strided 3D rhs APs over the padded image so only the
                # 448 real output pixels per row-tile stream (no junk
                # columns) and y is stored unpadded.
                for n in range(BPC):
                    if strip_x:
                        x_sb = x_static[n]
                    else:
                        x_sb = xp.tile([CIN, xcols], mm_dt)
                        nc.sync.dma_start(x_sb[:, :XSPLIT], x[n, :, :XSPLIT])
                        nc.sync.dma_start(x_sb[:, XSPLIT:], x[n, :, XSPLIT:])
                    xv = x_sb[:, : HP * WP].rearrange("p (h w) -> p h w", h=HP)
                    for m in range(COUT // 128):
                        o_sb = op.tile([128, NPIX], y_dt)
                        ydma = (nc.scalar.dma_start if ydge == "scalar"
                                else nc.sync.dma_start)
                        for ts in (range(0, 4), range(4, NT)):
                            ps = {t: pp.tile([128, NFREE], F32, name="ps",
                                             tag="ps") for t in ts}
                            for kh in range(KH):
                                for kw in range(KW):
                                    khw = kh * KW + kw
                                    lhsT = w_sb[:, khw * COUT + m * 128 : khw * COUT + m * 128 + 128]
                                    for t in ts:
                                        rhs = xv[:, t * ROWS_PER_TILE + kh : t * ROWS_PER_TILE + kh + ROWS_PER_TILE, kw : kw + W]
                                        nc.tensor.matmul(
                                            ps[t][:], lhsT, rhs,
                                            start=(khw == 0), stop=(khw == KH * KW - 1),
                                        )
                            for t in ts:
                                nc.vector.tensor_copy(
                                    o_sb[:, t * NFREE : (t + 1) * NFREE], ps[t][:])
                            if not strip_y:
                                lo, hi = ts[0] * NFREE, (ts[-1] + 1) * NFREE
                                ydma(y[n, m * 128 : (m + 1) * 128, lo:hi],
                                     o_sb[:, lo:hi])
                        last_o.append(o_sb)

            def body():
                for n in range(BPC):
                    if strip_x:
                        x_sb = x_static[n]
                    else:
                        x_sb = xp.tile([CIN, HP * WP], mm_dt)
                        if dma_mode == "split":
                            nc.sync.dma_start(x_sb[:, :XSPLIT], x[n, :, :XSPLIT])
                            nc.sync.dma_start(x_sb[:, XSPLIT:], x[n, :, XSPLIT:])
                        else:
                            nc.sync.dma_start(x_sb[:], x[n])
                    xv = x_sb[:].rearrange("p (h w) -> p h w", h=HP)
                    for m in range(COUT // 128):
                        o_sb = op.tile([128, NPIX], y_dt)
                        for t in range(NT):
                            ps = pp.tile([128, NFREE], F32, name="ps", tag="ps")
                            for kh in range(KH):
                                for kw in range(KW):
                                    khw = kh * KW + kw
                                    lhsT = w_sb[:, khw * COUT + m * 128 : khw * COUT + m * 128 + 128]
                                    rhs = xv[:, t * ROWS_PER_TILE + kh : t * ROWS_PER_TILE + kh + ROWS_PER_TILE, kw : kw + W]
                                    nc.tensor.matmul(
                                        ps[:], lhsT, rhs,
                                        start=(khw == 0), stop=(khw == KH * KW - 1),
                                    )
                            nc.vector.tensor_copy(o_sb[:, t * NFREE : (t + 1) * NFREE], ps[:])
                            # store as soon as a half-phase of copies is done
                            ydma = nc.scalar.dma_start if ydge == "scalar" else nc.sync.dma_start
                            if t == 3 and dma_mode == "split" and not strip_y:
                                ydma(
                                    y[n, m * 128 : (m + 1) * 128, : 4 * NFREE],
                                    o_sb[:, : 4 * NFREE],
                                )
                        if strip_y:
                            pass
                        elif dma_mode == "split":
                            ydma(
                                y[n, m * 128 : (m + 1) * 128, 4 * NFREE :],
                                o_sb[:, 4 * NFREE :],
                            )
                        else:
                            ydma(y[n, m * 128 : (m + 1) * 128, :], o_sb[:])
                last_o.append(o_sb)

            thebody = {"wstat": body_wstat, "ws448": body_ws448}.get(mode, body)
            if reps == 1:
                thebody()
            else:
                with tc.For_i(0, reps, 1,
                              hint_engines=(mybir.EngineType.PE,) if hint else ()):
                    thebody()
            if strip_y:
                nc.scalar.dma_start(y[0, :128, :], last_o[-1][:])
    if dedup:
        dedupe_ldweights(nc, verbose=verbose)
    nc.compile()
    nc._np_mm_dt = np_dt
    nc._mode = mode
    return nc


MODE = "ws448"

_NC_CACHE = None


def _get_nc():
    global _NC_CACHE
    if _NC_CACHE is None:
        _NC_CACHE = build_conv_bass(mode=MODE)
    return _NC_CACHE


def run_conv(inputs: np.ndarray, weight: np.ndarray, nc=None, **spmd_kwargs):
    """Returns (output, BassKernelResults)."""
    x = np.ascontiguousarray(np.asarray(inputs, dtype=np.float32))
    w = np.asarray(weight, dtype=np.float32)
    assert x.shape == (B, CIN, H, W) and w.shape == (COUT, CIN, KH, KW)

    nc = nc or _get_nc()
    np_dt = nc._np_mm_dt
    wstat = getattr(nc, "_mode", "base") == "wstat"

    xcols = XCOLS if wstat else HP * WP
    xpad = np.zeros((B, CIN, xcols), np_dt)
    xpad[:, :, : HP * WP].reshape(B, CIN, HP, WP)[:, :, 1 : H + 1, 1 : W + 1] = x
    # w_r[cin, khw*COUT + cout] = weight[cout, cin, kh, kw]
    wr = np.ascontiguousarray(
        w.transpose(1, 2, 3, 0).reshape(CIN, KH * KW * COUT).astype(np_dt)
    )

    in_maps = [
        {"x": xpad[c * BPC : (c + 1) * BPC], "w": wr} for c in range(NCORES)
    ]
    r = run_bass_kernel_spmd(nc, in_maps, core_ids=list(range(NCORES)), **spmd_kwargs)
    if wstat:
        out = np.concatenate(
            [np.asarray(r.results[c]["y"]).astype(np.float32)
             .reshape(BPC, COUT, H, WP)[:, :, :, :W] for c in range(NCORES)],
            axis=0,
        )
    else:
        out = np.concatenate(
            [np.asarray(r.results[c]["y"]).astype(np.float32).reshape(BPC, COUT, H, W)
             for c in range(NCORES)], axis=0
        )
    return np.ascontiguousarray(out), r


def kernel(inputs: np.ndarray, weight: np.ndarray) -> np.ndarray:
    out, _ = run_conv(inputs, weight)
    return out



# revision 8
# speedup vs baseline: 13.3322x; 9.7994x over previous
"""Conv2d 3x3 (pad=1, stride=1) on 8 TRN2 NeuronCores.

Input  (32, 128, 56, 56) f32, weight (256, 128, 3, 3) f32 -> out (32, 256, 56, 56) f32.

Strategy: data-parallel over batch (4 images per core). Per core, implicit GEMM:
for each of the 9 (kh, kw) taps, a [Cin=128 x Cout=128] stationary matmul against
a shifted window of the zero-padded input streams 448 output pixels (8 rows x 56)
per call, accumulating all 9 taps into one PSUM bank (fp32). Matmul operands are
fp16 (~3.6e-4 rel err from input quantization, fp32 accumulate). bf16 is the
same speed (2.4e-3 rel err); float32r ~2x slower; float32 4x slower (MM_DT).

Two hard-won facts (this machine):
- tile_legalize emits one LDWEIGHTS per MATMUL with no redundancy
  elimination; un-hidden that costs ~107ns per 448-col matmul (~35%).
  dedupe_ldweights() removes redundant loads post-legalize (weights are
  grouped tap-outer so 3-4 consecutive matmuls share the stationary
  operand); measured best-state per-iter dropped 151.6us -> 101.6us.
- mode="ws448" streams only the 448 real pixels per row-tile via strided
  3D APs (vs "wstat"'s 464-wide contiguous padded slabs): 94.1us PE floor.
  Sim (CoreSim no_exec): 103.1us single-shot, PE 94% busy.
"""

import sys

sys.path.insert(0, "/opt/trn_rl_repo")

import numpy as np

import concourse.bass as bass  # noqa: F401
import concourse.mybir as mybir
import concourse.tile as tile
from concourse import bacc
from concourse.bass_utils import run_bass_kernel_spmd

B, CIN, H, W = 32, 128, 56, 56
COUT, KH, KW = 256, 3, 3
NCORES = 8
BPC = B // NCORES  # images per core
HP, WP = H + 2, W + 2  # zero-padded
NPIX = H * W  # 3136
ROWS_PER_TILE = 8
NT = H // ROWS_PER_TILE  # 7 free-dim tiles of 448
NFREE = ROWS_PER_TILE * W  # 448

MM_DT = mybir.dt.float16
Y_DT = mybir.dt.float16  # output stored fp16, cast to f32 on host (exact)
F32 = mybir.dt.float32

WARMUP_MMS = 8

# wstat mode: accumulate in the PADDED output domain so every matmul rhs is
# one contiguous slab of the flat padded input. Output pixel (h, w) lives at
# flat q = h*WP + w; tap (kh, kw) contributes xflat[q + kh*WP + kw]. Junk
# columns (w in {56, 57}) are dropped on the host. 7 row-tiles of 8 rows:
# rhs slab for (t, kh, kw) = xflat[:, 8*t*WP + kh*WP + kw :][:464].
PFREE = 8 * WP  # 464 per tile, fits a 2KB PSUM bank (512 f32)
YPIX = H * WP  # 3248 padded output pixels per image
XCOLS = HP * WP + 4  # tail pad: t=6, kh=kw=2 reads up to 3366


def dedupe_ldweights(nc, verbose: bool = False):
    """Remove InstLdweights that reload the PE array with the exact weights
    already resident (same AP/tile_position/perf_mode), post-tile-legalize.

    tile_legalize splits every InstMatmult into Ldweights+Matmult with no
    redundancy elimination, so a weight-stationary group of G matmuls pays G
    serialized ~107ns array loads instead of 1. Only Ldweights with no
    semaphore waits/updates are dropped (sync edges must be preserved), and
    tracking resets at every non-Ldweights PE-array-state hazard
    (self-loading or transpose matmuls) and at block boundaries.
    """
    removed = kept = synced = 0
    for f in nc.m.functions:
        for b in f.blocks:
            insts = list(b.instructions)
            out = []
            last_w = None
            changed = False
            for i in insts:
                if isinstance(i, mybir.InstLdweights):
                    si = i.sync_info
                    has_sync = bool(si and (si.on_wait or si.on_update))
                    key = (i.ins[0].concise(), str(i.tile_position),
                           str(i.perf_mode), str(i.is_transpose))
                    if key == last_w and not has_sync:
                        removed += 1
                        changed = True
                        continue
                    if key == last_w:
                        synced += 1
                    last_w = key
                    kept += 1
                elif isinstance(i, mybir.InstMatmult):
                    if i.ldweights or i.is_transpose:
                        last_w = None
                out.append(i)
            if changed:
                b.instructions = out
    if verbose:
        print(f"dedupe_ldweights: removed={removed} kept={kept} "
              f"(kept-redundant-with-sync={synced})")
    return removed


def build_conv_bass(reps: int = 1, warmup: int = WARMUP_MMS, xbufs: int = 4,
                    hint: bool = False, mm_dt=None, ydge: str = "scalar",
                    ncores: int = NCORES, y_dt=None, dma_mode: str = "split",
                    strip: str = "", mode: str = "base", dedup: bool = True,
                    verbose: bool = False):
    mm_dt = mm_dt or MM_DT
    y_dt = y_dt or Y_DT
    np_dt = mybir.dt.np(mm_dt)
    wstat = mode == "wstat"
    xcols = XCOLS if wstat else HP * WP
    ypix = YPIX if wstat else NPIX
    nc = bacc.Bacc("TRN2", target_bir_lowering=False, debug=False, num_devices=ncores)
    x = nc.dram_tensor("x", [BPC, CIN, xcols], mm_dt, kind="ExternalInput").ap()
    w = nc.dram_tensor("w", [CIN, KH * KW * COUT], mm_dt, kind="ExternalInput").ap()
    y = nc.dram_tensor("y", [BPC, COUT, ypix], y_dt, kind="ExternalOutput").ap()
    strip_x = "x" in strip
    strip_y = "y" in strip

    with tile.TileContext(nc) as tc:
        with (
            tc.tile_pool(name="wp", bufs=1) as wp,
            tc.tile_pool(name="xp", bufs=xbufs) as xp,
            tc.tile_pool(name="op", bufs=2) as op,
            tc.tile_pool(name="pp", bufs=8, space="PSUM") as pp,
        ):
            w_sb = wp.tile([CIN, KH * KW * COUT], mm_dt)
            nc.sync.dma_start(w_sb[:], w)

            if warmup:
                # Warm the PE HAM clock gate while the first input DMA is in
                # flight: memset a scratch tile (no DMA dependency), then spin
                # matmuls on it into a scratch PSUM bank that is never read.
                scratch = wp.tile([128, 640], F32)
                nc.vector.memset(scratch[:], 0.0)
                sc = scratch[:].bitcast(mm_dt)  # >= [128, 640] for any dtype <= 4B
                ps_warm = pp.tile([128, 512], F32, name="ps_warm", tag="ps")
                for _ in range(warmup):
                    nc.tensor.matmul(ps_warm[:], sc[:, :128], sc[:, 128:640],
                                     start=True, stop=True)

            # x DMA split: rows [0, 34) cover tiles t=0..3 (rows t*8 .. t*8+9);
            # rows [34, 58) cover t=4..6. Tile tracks sub-tile ranges, so the
            # first matmuls start as soon as the first chunk lands.
            XSPLIT = (35 if wstat else 34) * WP

            def body_wstat():
                # Weight-stationary: taps outer, row-tiles inner. Each of the
                # 7 PSUM-resident tiles accumulates all 9 taps; weights swap
                # only at tap boundaries (consecutive same-weight matmuls
                # stream at the stationary rate). Two half-groups (4+3 tiles)
                # so copies/DMA of one overlap matmuls of the next.
                for n in range(BPC):
                    if strip_x:
                        x_sb = x_static[n]
                    else:
                        x_sb = xp.tile([CIN, xcols], mm_dt)
                        nc.sync.dma_start(x_sb[:, :XSPLIT], x[n, :, :XSPLIT])
                        nc.sync.dma_start(x_sb[:, XSPLIT:], x[n, :, XSPLIT:])
                    for m in range(COUT // 128):
                        o_sb = op.tile([128, ypix], y_dt)
                        ydma = (nc.scalar.dma_start if ydge == "scalar"
                                else nc.sync.dma_start)
                        for ts in (range(0, 4), range(4, NT)):
                            ps = {t: pp.tile([128, PFREE], F32, name="ps",
                                             tag="ps") for t in ts}
                            for kh in range(KH):
                                for kw in range(KW):
                                    khw = kh * KW + kw
                                    lhsT = w_sb[:, khw * COUT + m * 128 : khw * COUT + m * 128 + 128]
                                    for t in ts:
                                        s = (8 * t + kh) * WP + kw
                                        nc.tensor.matmul(
                                            ps[t][:], lhsT, x_sb[:, s : s + PFREE],
                                            start=(khw == 0), stop=(khw == KH * KW - 1),
                                        )
                            for t in ts:
                                nc.vector.tensor_copy(
                                    o_sb[:, t * PFREE : (t + 1) * PFREE], ps[t][:])
                            if not strip_y:
                                lo, hi = ts[0] * PFREE, (ts[-1] + 1) * PFREE
                                ydma(y[n, m * 128 : (m + 1) * 128, lo:hi],
                                     o_sb[:, lo:hi])
                        last_o.append(o_sb)

            last_o = []
            x_static = None
            if strip_x:
                x_static = []
                for n in range(BPC):
                    xs = xp.tile([CIN, xcols], mm_dt)
                    nc.sync.dma_start(xs[:], x[n])
                    x_static.append(xs)

            def body_ws448():
                # Weight-stationary like body_wstat (taps outer within a
                # half-group, dedupe collapses the per-tap LDWEIGHTS), but
                # with strided 3D rhs APs over the padded image so only the
                # 448 real output pixels per row-tile stream (no junk
                # columns) and y is stored unpadded. x lands in 3 chunks so
                # the first tap's matmuls start after ~12 rows instead of 34;
                # the trailing half-group stores per-tile so the post-last-
                # matmul tail is one copy + one small DMA, not three + large.
                XC1, XC2 = 12 * WP, 35 * WP
                for n in range(BPC):
                    if strip_x:
                        x_sb = x_static[n]
                    else:
                        x_sb = xp.tile([CIN, xcols], mm_dt)
                        nc.sync.dma_start(x_sb[:, :XC1], x[n, :, :XC1])
                        nc.sync.dma_start(x_sb[:, XC1:XC2], x[n, :, XC1:XC2])
                        nc.sync.dma_start(x_sb[:, XC2:], x[n, :, XC2:])
                    xv = x_sb[:, : HP * WP].rearrange("p (h w) -> p h w", h=HP)
                    for m in range(COUT // 128):
                        o_sb = op.tile([128, NPIX], y_dt)
                        ydma = (nc.scalar.dma_start if ydge == "scalar"
                                else nc.sync.dma_start)
                        for ts in (range(0, 4), range(4, NT)):
                            ps = {t: pp.tile([128, NFREE], F32, name="ps",
                                             tag="ps") for t in ts}
                            for kh in range(KH):
                                for kw in range(KW):
                                    khw = kh * KW + kw
                                    lhsT = w_sb[:, khw * COUT + m * 128 : khw * COUT + m * 128 + 128]
                                    for t in ts:
                                        rhs = xv[:, t * ROWS_PER_TILE + kh : t * ROWS_PER_TILE + kh + ROWS_PER_TILE, kw : kw + W]
                                        nc.tensor.matmul(
                                            ps[t][:], lhsT, rhs,
                                            start=(khw == 0), stop=(khw == KH * KW - 1),
                                        )
                            if ts[0] == 0:
                                for t in ts:
                                    nc.vector.tensor_copy(
                                        o_sb[:, t * NFREE : (t + 1) * NFREE], ps[t][:])
                                if not strip_y:
                                    lo, hi = ts[0] * NFREE, (ts[-1] + 1) * NFREE
                                    ydma(y[n, m * 128 : (m + 1) * 128, lo:hi],
                                         o_sb[:, lo:hi])
                            else:
                                for t in ts:
                                    nc.vector.tensor_copy(
                                        o_sb[:, t * NFREE : (t + 1) * NFREE], ps[t][:])
                                    if not strip_y:
                                        ydma(y[n, m * 128 : (m + 1) * 128,
                                               t * NFREE : (t + 1) * NFREE],
                                             o_sb[:, t * NFREE : (t + 1) * NFREE])
                        last_o.append(o_sb)

            def body():
                for n in range(BPC):
                    if strip_x:
                        x_sb = x_static[n]
                    else:
                        x_sb = xp.tile([CIN, HP * WP], mm_dt)
                        if dma_mode == "split":
                            nc.sync.dma_start(x_sb[:, :XSPLIT], x[n, :, :XSPLIT])
                            nc.sync.dma_start(x_sb[:, XSPLIT:], x[n, :, XSPLIT:])
                        else:
                            nc.sync.dma_start(x_sb[:], x[n])
                    xv = x_sb[:].rearrange("p (h w) -> p h w", h=HP)
                    for m in range(COUT // 128):
                        o_sb = op.tile([128, NPIX], y_dt)
                        for t in range(NT):
                            ps = pp.tile([128, NFREE], F32, name="ps", tag="ps")
                            for kh in range(KH):
                                for kw in range(KW):
                                    khw = kh * KW + kw
                                    lhsT = w_sb[:, khw * COUT + m * 128 : khw * COUT + m * 128 + 128]
                                    rhs = xv[:, t * ROWS_PER_TILE + kh : t * ROWS_PER_TILE + kh + ROWS_PER_TILE, kw : kw + W]
                                    nc.tensor.matmul(
                                        ps[:], lhsT, rhs,
                                        start=(khw == 0), stop=(khw == KH * KW - 1),
                                    )
                            nc.vector.tensor_copy(o_sb[:, t * NFREE : (t + 1) * NFREE], ps[:])
                            # store as soon as a half-phase of copies is done
                            ydma = nc.scalar.dma_start if ydge == "scalar" else nc.sync.dma_start
                            if t == 3 and dma_mode == "split" and not strip_y:
                                ydma(
                                    y[n, m * 128 : (m + 1) * 128, : 4 * NFREE],
                                    o_sb[:, : 4 * NFREE],
                                )
                        if strip_y:
                            pass
                        elif dma_mode == "split":
                            ydma(
                                y[n, m * 128 : (m + 1) * 128, 4 * NFREE :],
                                o_sb[:, 4 * NFREE :],
                            )
                        else:
                            ydma(y[n, m * 128 : (m + 1) * 128, :], o_sb[:])
                last_o.append(o_sb)

            thebody = {"wstat": body_wstat, "ws448": body_ws448}.get(mode, body)
            if reps == 1:
                thebody()
            else:
                with tc.For_i(0, reps, 1,
                              hint_engines=(mybir.EngineType.PE,) if hint else ()):
                    thebody()
            if strip_y:
                nc.scalar.dma_start(y[0, :128, :], last_o[-1][:])
    if dedup:
        dedupe_ldweights(nc, verbose=verbose)
    nc.compile()
    nc._np_mm_dt = np_dt
    nc._mode = mode
    return nc


MODE = "ws448"

_NC_CACHE = None


def _get_nc():
    global _NC_CACHE
    if _NC_CACHE is None:
        _NC_CACHE = build_conv_bass(mode=MODE)
    return _NC_CACHE


def run_conv(inputs: np.ndarray, weight: np.ndarray, nc=None, **spmd_kwargs):
    """Returns (output, BassKernelResults)."""
    x = np.ascontiguousarray(np.asarray(inputs, dtype=np.float32))
    w = np.asarray(weight, dtype=np.float32)
    assert x.shape == (B, CIN, H, W) and w.shape == (COUT, CIN, KH, KW)

    nc = nc or _get_nc()
    np_dt = nc._np_mm_dt
    wstat = getattr(nc, "_mode", "base") == "wstat"

    xcols = XCOLS if wstat else HP * WP
    xpad = np.zeros((B, CIN, xcols), np_dt)
    xpad[:, :, : HP * WP].reshape(B, CIN, HP, WP)[:, :, 1 : H + 1, 1 : W + 1] = x
    # w_r[cin, khw*COUT + cout] = weight[cout, cin, kh, kw]
    wr = np.ascontiguousarray(
        w.transpose(1, 2, 3, 0).reshape(CIN, KH * KW * COUT).astype(np_dt)
    )

    in_maps = [
        {"x": xpad[c * BPC : (c + 1) * BPC], "w": wr} for c in range(NCORES)
    ]
    r = run_bass_kernel_spmd(nc, in_maps, core_ids=list(range(NCORES)), **spmd_kwargs)
    if wstat:
        out = np.concatenate(
            [np.asarray(r.results[c]["y"]).astype(np.float32)
             .reshape(BPC, COUT, H, WP)[:, :, :, :W] for c in range(NCORES)],
            axis=0,
        )
    else:
        out = np.concatenate(
            [np.asarray(r.results[c]["y"]).astype(np.float32).reshape(BPC, COUT, H, W)
             for c in range(NCORES)], axis=0
        )
    return np.ascontiguousarray(out), r


def kernel(inputs: np.ndarray, weight: np.ndarray) -> np.ndarray:
    out, _ = run_conv(inputs, weight)
    return out

